# revision 25
# baseline (speedup 1.0000x reference)
"""AttentionFlowLayer (BiDAF-style) Trainium2 kernel, 8 NeuronCores.

Sharding: data-parallel over batch N=16 -> 2 batches per core, weights
replicated, no collectives.  Host preprocessing is pure layout: inputs
cast to bf16 and rearranged partition-major (so every DMA segment is
>=2KB contiguous), context also supplied pre-transposed (cT), query
padded with a ones column.  Device outputs are partition-major too and
the host rearranges them back; output chunk0 is the bf16 context
verbatim, assembled on host (the cT load replaces the chunk0 store, so
total HBM bytes match a store-chunk0 design).

The score matrix is computed TRANSPOSED (S_T[j,i], query on partitions)
with the tiny qv blocks stationary and cT streaming.  This kills both
the per-tile qw rank-1 matmuls (qw[j] rides the ACT exp bias, per-
partition in this layout) and all Ap transposes (exp writes A_T straight
into the layout the c2q matmul needs as lhsT):

  qv[d,j]  = wm[d]*qT[d,j] + wc[d]           (affine fold, one DVE op)
  S_T[j,i] = sum_d qv[d,j]*cT[d,i]   (+ qw[j] via exp bias)
             (the wc term contributes sum_d wc[d]*cT[d,i] = cw[i])
  A_T      = exp(S_T + qw)   [j-part, i]  (ACT, 512-wide psum slabs)
  m0[i]    = max_j A_T = exp(max_j S[i,j])   (TT-max of the two j-chunks,
             gpsimd partition-max, 16 thin PE transposes back to i-part)
  c2q psum = A_T-blocks @ [q | 1] -> cols 0..255 = A@q, col 256 = Z_i
  c2q      = (A @ q) / Z_i   (normalize folded into the psum->sbuf copy,
             split between ACT and DVE to balance the engines)
  q2c      = (sum_i m0[i]*c[i,:]) / sum_i m0[i]
  device stores: out12 = [c2q | c*c2q] (2KB rows), out3 = c*q2c.

Emission is slab-staggered with one slab of lookahead; batch 1's S
phases interleave with batch 0's tail, and each batch's q2c chain is
pulled ahead of its last two c2q slabs so the chunk3 stores overlap
them.  All DMAs ride the SP hwdge ring; loads are emitted first.
"""

import numpy as np

N, LC, LQ, D = 16, 2048, 256, 256
NCORES = 8
NB = N // NCORES      # batches per core
P = 128
T = LC // P           # context tiles per batch (16)
JT = LQ // P          # query partition tiles (2)
DC = D // P           # d chunks (2)
OG = 4                # tiles per slab (exp batch, DMA group, elementwise)
SW = OG * P           # slab width in i (512)
NS = T // OG          # slabs per batch (4)

_cache = {}


def _build():
    import concourse.mybir as mybir
    from concourse import bacc, bass_isa
    from concourse.tile import TileContext
    from concourse.masks import make_identity

    f32 = mybir.dt.float32
    bf16 = mybir.dt.bfloat16
    EXP = mybir.ActivationFunctionType.Exp
    COPY = mybir.ActivationFunctionType.Copy
    AX = mybir.AxisListType.X
    MULT = mybir.AluOpType.mult
    ADD = mybir.AluOpType.add

    nc = bacc.Bacc("TRN2")
    # partition-major layouts: [.., P, ..] with >=2KB contiguous per partition
    c_in = nc.dram_tensor("c_pm", (NB, P, T, D), bf16, kind="ExternalInput")
    ct_in = nc.dram_tensor("ct_pm", (NB, P, DC, LC), bf16, kind="ExternalInput")
    q_in = nc.dram_tensor("q_pm", (NB, P, JT, D + 1), bf16, kind="ExternalInput")
    w_in = nc.dram_tensor("W", (3 * D,), f32, kind="ExternalInput")
    out12 = nc.dram_tensor("out12", (NB, P, T, 2 * D), bf16, kind="ExternalOutput")
    out3 = nc.dram_tensor("out3", (NB, P, T, D), bf16, kind="ExternalOutput")

    with TileContext(nc) as tc:
        with (
            tc.tile_pool(name="const", bufs=1) as constp,
            tc.tile_pool(name="qpool", bufs=2) as qpool,
            tc.tile_pool(name="cfull", bufs=2) as cfp,
            tc.tile_pool(name="perb", bufs=2) as perb,
            tc.tile_pool(name="gbig", bufs=2) as gp,
            tc.tile_pool(name="small", bufs=8) as smallp,
            tc.tile_pool(name="ps_t", bufs=1, space="PSUM") as ps_tp,
            tc.tile_pool(name="ps_s", bufs=3, space="PSUM") as ps_sp,
            tc.tile_pool(name="ps_cq", bufs=2, space="PSUM") as ps_cqp,
            tc.tile_pool(name="ps_sm", bufs=1, space="PSUM") as ps_smp,
            tc.tile_pool(name="ps_m0", bufs=1, space="PSUM") as ps_m0p,
        ):
            ident = constp.tile([P, P], bf16, tag="ident")
            make_identity(nc, ident)
            ones_row = constp.tile([1, P], bf16, tag="ones_row")
            nc.vector.memset(ones_row, 1.0)
            ones_col = constp.tile([P, 1], bf16, tag="ones_col")
            nc.vector.memset(ones_col, 1.0)
            # W columns: [wc0 wc1 wq0 wq1 wm0 wm1], chunk c covers d=c*128..+127
            wcols = constp.tile([P, 6], f32, tag="wcols")

            # ---- all input loads up-front on the SP hwdge ring
            nc.sync.dma_start(wcols, w_in[:].rearrange("(c p) -> p c", p=P))
            qins, cins, cts = [], [], []
            for b in range(NB):
                qin = qpool.tile([P, JT, D + 1], bf16, tag="qin")
                nc.sync.dma_start(qin, q_in[b])
                qins.append(qin)
            cT0 = cfp.tile([P, DC, LC], bf16, tag="ct16", name="ct0")
            nc.sync.dma_start(cT0, ct_in[0])
            cts.append(cT0)
            cin0 = cfp.tile([P, T, D], bf16, tag="cin", name="cin0")
            nc.sync.dma_start(cin0, c_in[0])
            cins.append(cin0)
            cT1 = cfp.tile([P, DC, LC], bf16, tag="ct16", name="ct1")
            nc.sync.dma_start(cT1, ct_in[1])
            cts.append(cT1)
            cin1 = cfp.tile([P, T, D], bf16, tag="cin", name="cin1")
            nc.sync.dma_start(cin1, c_in[1])
            cins.append(cin1)

            wq16 = constp.tile([P, 2], bf16, tag="wq16")
            nc.vector.tensor_copy(wq16, wcols[:, 2:4])

            # ---- q prep for BOTH batches up-front: all PE transposes first,
            # then the DVE copies/affines, then the qw matmuls (minimizes
            # cross-engine ping-pong before the first S matmul)
            qvs, qwcs = [], []
            ps_qs, qT16s = [], []
            for b in range(NB):
                qin = qins[b]
                ps_q = ps_tp.tile([P, DC, JT, P], bf16, tag="pst", name=f"ps_q{b}")
                for c in range(DC):
                    for jt in range(JT):
                        nc.tensor.transpose(
                            ps_q[:, c, jt, :], qin[:, jt, c * P:(c + 1) * P], ident
                        )
                ps_qs.append(ps_q)
            for b in range(NB):
                qT16 = qpool.tile([P, DC, LQ], bf16, tag="qT16", name=f"qT16_{b}")
                nc.vector.tensor_copy(qT16, ps_qs[b])
                qT16s.append(qT16)
                qv = qpool.tile([P, DC, LQ], bf16, tag="qv", name=f"qv{b}")
                for c in range(DC):
                    nc.vector.tensor_scalar(
                        qv[:, c, :], qT16[:, c, :],
                        wcols[:, 4 + c:5 + c], wcols[:, c:c + 1], MULT, ADD,
                    )
                qvs.append(qv)
            for b in range(NB):
                qwc = smallp.tile([P, JT], f32, tag="qwc", name=f"qwc{b}")
                qwcs.append(qwc)

            def emit_qw(b):
                # deferred so the first S matmuls (which only need qv/cT)
                # are not queued behind these on the PE
                qT16 = qT16s[b]
                ps_qw = ps_smp.tile([P, JT], f32, tag="sm", name=f"ps_qw{b}")
                for jh in range(JT):
                    for c in range(DC):
                        nc.tensor.matmul(
                            ps_qw[:, jh:jh + 1],
                            lhsT=qT16[:, c, jh * P:(jh + 1) * P],
                            rhs=wq16[:, c:c + 1],
                            start=(c == 0), stop=(c == DC - 1),
                        )
                nc.vector.tensor_copy(qwcs[b], ps_qw)

            def batch_ctx(b):
                qin = qins[b]
                cin = cins[b]
                cT16 = cts[b]
                qv = qvs[b]
                qwc = qwcs[b]

                # per-batch staging (resident for the batch)
                AT = perb.tile([P, JT, LC], bf16, tag="at")
                Amax = perb.tile([P, LC], bf16, tag="amax")
                m0bc = perb.tile([P, LC], bf16, tag="m0bc")
                m016 = perb.tile([P, T], bf16, tag="m016")
                invZ = perb.tile([P, T], f32, tag="invz")
                g12 = gp.tile([P, T, 2 * D], bf16, tag="g12")
                g3 = gp.tile([P, T, D], bf16, tag="g3")
                ps_m0 = ps_m0p.tile([P, T], f32, tag="m0")

                # B(s): S_T matmuls + exp (qw via per-partition bias)
                def phase_B(s):
                    i0 = s * SW
                    for jc in range(JT):
                        ps_st = ps_sp.tile([P, SW], f32, tag="ps_s")
                        for c in range(DC):
                            nc.tensor.matmul(
                                ps_st,
                                lhsT=qv[:, c, jc * P:(jc + 1) * P],
                                rhs=cT16[:, c, i0:i0 + SW],
                                start=(c == 0), stop=(c == DC - 1),
                            )
                        nc.scalar.activation(
                            AT[:, jc, i0:i0 + SW], ps_st, EXP,
                            bias=qwc[:, jc:jc + 1],
                        )

                # C(s): m0 for the slab: jc-pair max, partition max, 4 thin
                # transposes to put m0 on i-partitions
                def phase_C(s):
                    i0 = s * SW
                    nc.vector.tensor_max(
                        Amax[:, i0:i0 + SW],
                        AT[:, 0, i0:i0 + SW], AT[:, 1, i0:i0 + SW],
                    )
                    nc.gpsimd.partition_all_reduce(
                        m0bc[:, i0:i0 + SW], Amax[:, i0:i0 + SW],
                        128, bass_isa.ReduceOp.max,
                    )
                    for k in range(OG):
                        t = s * OG + k
                        nc.tensor.matmul(
                            ps_m0[:, t:t + 1],
                            lhsT=m0bc[0:1, t * P:(t + 1) * P],
                            rhs=ones_col[0:1, :],
                            start=True, stop=True,
                        )

                # D(s): c2q matmuls (Z column), recip, normalized copy
                # (ACT/DVE split), combined chunk1+2 store
                def phase_D(s):
                    t0 = s * OG
                    for k in range(OG):
                        t = t0 + k
                        ps_cq = ps_cqp.tile([P, LQ + 1], f32, tag="cq")
                        for jc in range(JT):
                            nc.tensor.matmul(
                                ps_cq,
                                lhsT=AT[:, jc, t * P:(t + 1) * P],
                                rhs=qin[:, jc, :],
                                start=(jc == 0), stop=(jc == JT - 1),
                            )
                        nc.vector.reciprocal(invZ[:, t:t + 1], ps_cq[:, D:D + 1])
                        if k % 2 == 0:
                            nc.scalar.activation(
                                g12[:, t, 0:D], ps_cq[:, 0:D], COPY,
                                scale=invZ[:, t:t + 1],
                            )
                        else:
                            nc.vector.tensor_scalar_mul(
                                g12[:, t, 0:D], ps_cq[:, 0:D], invZ[:, t:t + 1]
                            )
                    nc.vector.tensor_mul(
                        g12[:, t0:t0 + OG, D:2 * D], cin[:, t0:t0 + OG, :],
                        g12[:, t0:t0 + OG, 0:D],
                    )
                    nc.sync.dma_start(
                        out12[b, :, t0:t0 + OG, :], g12[:, t0:t0 + OG, :]
                    )

                def tail_q2c():
                    # q2c chain (needs m0 of all 16 tiles)
                    nc.vector.tensor_copy(m016, ps_m0)
                    ebrow = smallp.tile([P, 1], f32, tag="ebrow")
                    nc.vector.reduce_sum(ebrow, m016, axis=AX)
                    ebrow16 = smallp.tile([P, 1], bf16, tag="ebrow16")
                    nc.vector.tensor_copy(ebrow16, ebrow)
                    ps_zb = ps_smp.tile([1, 1], f32, tag="sm")
                    nc.tensor.matmul(
                        ps_zb, lhsT=ebrow16, rhs=ones_col, start=True, stop=True
                    )
                    zb = smallp.tile([1, 1], f32, tag="zbs")
                    nc.vector.tensor_copy(zb, ps_zb)
                    inv_zb = smallp.tile([1, 1], f32, tag="invzb")
                    nc.vector.reciprocal(inv_zb, zb)
                    ps_q2c = ps_smp.tile([1, D], f32, tag="sm")
                    for t in range(T):
                        nc.tensor.matmul(
                            ps_q2c, lhsT=m016[:, t:t + 1], rhs=cin[:, t, :],
                            start=(t == 0), stop=(t == T - 1),
                        )
                    q2cn16 = smallp.tile([1, D], bf16, tag="q2cn")
                    nc.scalar.activation(q2cn16, ps_q2c, COPY, scale=inv_zb)
                    ps_bc = ps_smp.tile([P, D], f32, tag="sm")
                    nc.tensor.matmul(
                        ps_bc, lhsT=ones_row, rhs=q2cn16, start=True, stop=True
                    )
                    q2cb16 = perb.tile([P, D], bf16, tag="q2cb")
                    nc.vector.tensor_copy(q2cb16, ps_bc)
                    return q2cb16

                def tail_g3(q2cb16, s):
                    t0 = s * OG
                    nc.vector.tensor_mul(
                        g3[:, t0:t0 + OG, :], cin[:, t0:t0 + OG, :],
                        q2cb16[:, None, :].to_broadcast((P, OG, D)),
                    )
                    nc.sync.dma_start(
                        out3[b, :, t0:t0 + OG, :], g3[:, t0:t0 + OG, :]
                    )

                return phase_B, phase_C, phase_D, tail_q2c, tail_g3

            # ---- slab-staggered emission; each batch's q2c chain is pulled
            # ahead of its last two c2q slabs; batch 1's S phases interleave
            # with batch 0's tail.
            B0, C0, D0, Q0, G0 = batch_ctx(0)
            B1, C1, D1, Q1, G1 = batch_ctx(1)
            emit_qw(0); B0(0); B0(1); emit_qw(1); C0(0); B0(2); C0(1); D0(0)
            B0(3); C0(2); D0(1); C0(3)
            q2cb0 = Q0()
            D0(2); G0(q2cb0, 0); D0(3); G0(q2cb0, 1)
            B1(0); G0(q2cb0, 2); B1(1); C1(0); G0(q2cb0, 3)
            B1(2); C1(1); D1(0); B1(3); C1(2); D1(1); C1(3)
            q2cb1 = Q1()
            D1(2); G1(q2cb1, 0); G1(q2cb1, 1); D1(3); G1(q2cb1, 2); G1(q2cb1, 3)

    nc.compile()
    return nc


def _get_nc():
    if "nc" not in _cache:
        _cache["nc"] = _build()
    return _cache["nc"]


def run(emb_context, emb_query, W, trace=False, **kwargs):
    import ml_dtypes
    from concourse.bass_utils import run_bass_kernel_spmd

    nc = _get_nc()
    bf = ml_dtypes.bfloat16
    c16 = np.asarray(emb_context, dtype=np.float32).astype(bf)
    # partition-major layouts (pure layout transforms)
    c_pm = np.ascontiguousarray(
        c16.reshape(N, T, P, D).transpose(0, 2, 1, 3)
    )  # (N, P, T, D)
    ct = c16.transpose(0, 2, 1)  # (N, D, LC)
    ct_pm = np.ascontiguousarray(
        ct.reshape(N, DC, P, LC).transpose(0, 2, 1, 3)
    )  # (N, P, DC, LC)
    eq = np.asarray(emb_query, dtype=np.float32).astype(bf)
    q_p = np.concatenate([eq, np.ones((N, LQ, 1), dtype=bf)], axis=2)
    q_pm = np.ascontiguousarray(
        q_p.reshape(N, JT, P, D + 1).transpose(0, 2, 1, 3)
    )  # (N, P, JT, D+1)
    W = np.asarray(W, dtype=np.float32)
    in_maps = [
        {
            "c_pm": np.ascontiguousarray(c_pm[c * NB:(c + 1) * NB]),
            "ct_pm": np.ascontiguousarray(ct_pm[c * NB:(c + 1) * NB]),
            "q_pm": np.ascontiguousarray(q_pm[c * NB:(c + 1) * NB]),
            "W": W,
        }
        for c in range(NCORES)
    ]
    res = run_bass_kernel_spmd(
        nc, in_maps, core_ids=list(range(NCORES)), trace=trace, **kwargs
    )
    # assemble: chunk0 = bf16 context verbatim; device chunks back to
    # row-major (pure layout)
    full = np.empty((N, LC, 4 * D), dtype=np.float32)
    full[:, :, 0:D] = c16.astype(np.float32)
    o12 = np.stack([np.asarray(r["out12"]) for r in res.results])  # (8,NB,P,T,2D)
    o3 = np.stack([np.asarray(r["out3"]) for r in res.results])
    o12 = o12.reshape(N, P, T, 2 * D).transpose(0, 2, 1, 3).reshape(N, LC, 2 * D)
    o3 = o3.reshape(N, P, T, D).transpose(0, 2, 1, 3).reshape(N, LC, D)
    full[:, :, D:3 * D] = o12.astype(np.float32)
    full[:, :, 3 * D:] = o3.astype(np.float32)
    return full, res


def kernel(emb_context, emb_query, W):
    out, _ = run(emb_context, emb_query, W, trace=False)
    return out


# revision 26
# speedup vs baseline: 1.0401x; 1.0401x over previous
"""AttentionFlowLayer (BiDAF-style) Trainium2 kernel, 8 NeuronCores.

Sharding: data-parallel over batch N=16 -> 2 batches per core, weights
replicated, no collectives.  Host preprocessing is pure layout: inputs
cast to bf16 and rearranged partition-major (so every DMA segment is
>=2KB contiguous), context also supplied pre-transposed (cT), query
padded with a ones column.  Device outputs are partition-major too and
the host rearranges them back; output chunk0 is the bf16 context
verbatim, assembled on host (the cT load replaces the chunk0 store, so
total HBM bytes match a store-chunk0 design).

The score matrix is computed TRANSPOSED (S_T[j,i], query on partitions)
with the tiny qv blocks stationary and cT streaming.  This kills both
the per-tile qw rank-1 matmuls (qw[j] rides the ACT exp bias, per-
partition in this layout) and all Ap transposes (exp writes A_T straight
into the layout the c2q matmul needs as lhsT):

  qv[d,j]  = wm[d]*qT[d,j] + wc[d]           (affine fold, one DVE op)
  S_T[j,i] = sum_d qv[d,j]*cT[d,i]   (+ qw[j] via exp bias)
             (the wc term contributes sum_d wc[d]*cT[d,i] = cw[i])
  A_T      = exp(S_T + qw)   [j-part, i]  (ACT, 512-wide psum slabs)
  m0[i]    = max_j A_T = exp(max_j S[i,j])   (TT-max of the two j-chunks,
             gpsimd partition-max, 16 thin PE transposes back to i-part)
  c2q psum = A_T-blocks @ [q | 1] -> cols 0..255 = A@q, col 256 = Z_i
  c2q      = (A @ q) / Z_i   (normalize folded into the psum->sbuf copy,
             split between ACT and DVE to balance the engines)
  q2c      = (sum_i m0[i]*c[i,:]) / sum_i m0[i]
  device stores: out12 = [c2q | c*c2q] (2KB rows), out3 = c*q2c.

Emission is slab-staggered with one slab of lookahead; batch 1's S
phases interleave with batch 0's tail, and each batch's q2c chain is
pulled ahead of its last two c2q slabs so the chunk3 stores overlap
them.  All DMAs ride the SP hwdge ring; loads are emitted first.
"""

import numpy as np

N, LC, LQ, D = 16, 2048, 256, 256
NCORES = 8
NB = N // NCORES      # batches per core
P = 128
T = LC // P           # context tiles per batch (16)
JT = LQ // P          # query partition tiles (2)
DC = D // P           # d chunks (2)
OG = 4                # tiles per slab (exp batch, DMA group, elementwise)
SW = OG * P           # slab width in i (512)
NS = T // OG          # slabs per batch (4)

_cache = {}


def _build():
    import concourse.mybir as mybir
    from concourse import bacc, bass_isa
    from concourse.tile import TileContext
    from concourse.masks import make_identity

    f32 = mybir.dt.float32
    bf16 = mybir.dt.bfloat16
    EXP = mybir.ActivationFunctionType.Exp
    COPY = mybir.ActivationFunctionType.Copy
    AX = mybir.AxisListType.X
    MULT = mybir.AluOpType.mult
    ADD = mybir.AluOpType.add

    nc = bacc.Bacc("TRN2")
    # partition-major layouts: [.., P, ..] with >=2KB contiguous per partition
    c_in = nc.dram_tensor("c_pm", (NB, P, T, D), bf16, kind="ExternalInput")
    ct_in = nc.dram_tensor("ct_pm", (NB, P, DC, LC), bf16, kind="ExternalInput")
    q_in = nc.dram_tensor("q_pm", (NB, P, JT, D + 1), bf16, kind="ExternalInput")
    w_in = nc.dram_tensor("W", (3 * D,), f32, kind="ExternalInput")
    out12 = nc.dram_tensor("out12", (NB, P, T, 2 * D), bf16, kind="ExternalOutput")
    out3 = nc.dram_tensor("out3", (NB, P, T, D), bf16, kind="ExternalOutput")

    with TileContext(nc) as tc:
        with (
            tc.tile_pool(name="const", bufs=1) as constp,
            tc.tile_pool(name="qpool", bufs=2) as qpool,
            tc.tile_pool(name="cfull", bufs=2) as cfp,
            tc.tile_pool(name="perb", bufs=2) as perb,
            tc.tile_pool(name="gbig", bufs=2) as gp,
            tc.tile_pool(name="small", bufs=8) as smallp,
            tc.tile_pool(name="ps_t", bufs=1, space="PSUM") as ps_tp,
            tc.tile_pool(name="ps_s", bufs=3, space="PSUM") as ps_sp,
            tc.tile_pool(name="ps_cq", bufs=2, space="PSUM") as ps_cqp,
            tc.tile_pool(name="ps_sm", bufs=1, space="PSUM") as ps_smp,
            tc.tile_pool(name="ps_m0", bufs=1, space="PSUM") as ps_m0p,
        ):
            ident = constp.tile([P, P], bf16, tag="ident")
            make_identity(nc, ident)
            ones_row = constp.tile([1, P], bf16, tag="ones_row")
            nc.vector.memset(ones_row, 1.0)
            ones_col = constp.tile([P, 1], bf16, tag="ones_col")
            nc.vector.memset(ones_col, 1.0)
            # W columns: [wc0 wc1 wq0 wq1 wm0 wm1], chunk c covers d=c*128..+127
            wcols = constp.tile([P, 6], f32, tag="wcols")

            # ---- all input loads up-front on the SP hwdge ring
            nc.scalar.dma_start(wcols, w_in[:].rearrange("(c p) -> p c", p=P))
            qins, cins, cts = [], [], []
            for b in range(NB):
                qin = qpool.tile([P, JT, D + 1], bf16, tag="qin")
                nc.scalar.dma_start(qin, q_in[b])
                qins.append(qin)
            cT0 = cfp.tile([P, DC, LC], bf16, tag="ct16", name="ct0")
            nc.sync.dma_start(cT0, ct_in[0])
            cts.append(cT0)
            cin0 = cfp.tile([P, T, D], bf16, tag="cin", name="cin0")
            nc.sync.dma_start(cin0, c_in[0])
            cins.append(cin0)
            cT1 = cfp.tile([P, DC, LC], bf16, tag="ct16", name="ct1")
            nc.sync.dma_start(cT1, ct_in[1])
            cts.append(cT1)
            cin1 = cfp.tile([P, T, D], bf16, tag="cin", name="cin1")
            nc.sync.dma_start(cin1, c_in[1])
            cins.append(cin1)

            wq16 = constp.tile([P, 2], bf16, tag="wq16")
            nc.vector.tensor_copy(wq16, wcols[:, 2:4])

            # ---- q prep for BOTH batches up-front: all PE transposes first,
            # then the DVE copies/affines, then the qw matmuls (minimizes
            # cross-engine ping-pong before the first S matmul)
            qvs, qwcs = [], []
            ps_qs, qT16s = [], []
            for b in range(NB):
                qin = qins[b]
                ps_q = ps_tp.tile([P, DC, JT, P], bf16, tag="pst", name=f"ps_q{b}")
                for c in range(DC):
                    for jt in range(JT):
                        nc.tensor.transpose(
                            ps_q[:, c, jt, :], qin[:, jt, c * P:(c + 1) * P], ident
                        )
                ps_qs.append(ps_q)
            for b in range(NB):
                qT16 = qpool.tile([P, DC, LQ], bf16, tag="qT16", name=f"qT16_{b}")
                nc.vector.tensor_copy(qT16, ps_qs[b])
                qT16s.append(qT16)
                qv = qpool.tile([P, DC, LQ], bf16, tag="qv", name=f"qv{b}")
                for c in range(DC):
                    nc.vector.tensor_scalar(
                        qv[:, c, :], qT16[:, c, :],
                        wcols[:, 4 + c:5 + c], wcols[:, c:c + 1], MULT, ADD,
                    )
                qvs.append(qv)
            for b in range(NB):
                qwc = smallp.tile([P, JT], f32, tag="qwc", name=f"qwc{b}")
                qwcs.append(qwc)

            def emit_qw(b):
                # deferred so the first S matmuls (which only need qv/cT)
                # are not queued behind these on the PE
                qT16 = qT16s[b]
                ps_qw = ps_smp.tile([P, JT], f32, tag="sm", name=f"ps_qw{b}")
                for jh in range(JT):
                    for c in range(DC):
                        nc.tensor.matmul(
                            ps_qw[:, jh:jh + 1],
                            lhsT=qT16[:, c, jh * P:(jh + 1) * P],
                            rhs=wq16[:, c:c + 1],
                            start=(c == 0), stop=(c == DC - 1),
                        )
                nc.vector.tensor_copy(qwcs[b], ps_qw)

            def batch_ctx(b):
                qin = qins[b]
                cin = cins[b]
                cT16 = cts[b]
                qv = qvs[b]
                qwc = qwcs[b]

                # per-batch staging (resident for the batch)
                AT = perb.tile([P, JT, LC], bf16, tag="at")
                Amax = perb.tile([P, LC], bf16, tag="amax")
                m0bc = perb.tile([P, LC], bf16, tag="m0bc")
                m016 = perb.tile([P, T], bf16, tag="m016")
                invZ = perb.tile([P, T], f32, tag="invz")
                g12 = gp.tile([P, T, 2 * D], bf16, tag="g12")
                g3 = gp.tile([P, T, D], bf16, tag="g3")
                ps_m0 = ps_m0p.tile([P, T], f32, tag="m0")

                # B(s): S_T matmuls + exp (qw via per-partition bias)
                def phase_B(s):
                    i0 = s * SW
                    for jc in range(JT):
                        ps_st = ps_sp.tile([P, SW], f32, tag="ps_s")
                        for c in range(DC):
                            nc.tensor.matmul(
                                ps_st,
                                lhsT=qv[:, c, jc * P:(jc + 1) * P],
                                rhs=cT16[:, c, i0:i0 + SW],
                                start=(c == 0), stop=(c == DC - 1),
                            )
                        nc.scalar.activation(
                            AT[:, jc, i0:i0 + SW], ps_st, EXP,
                            bias=qwc[:, jc:jc + 1],
                        )

                # C(s): m0 for the slab: jc-pair max, partition max, 4 thin
                # transposes to put m0 on i-partitions
                def phase_C(s):
                    i0 = s * SW
                    nc.vector.tensor_max(
                        Amax[:, i0:i0 + SW],
                        AT[:, 0, i0:i0 + SW], AT[:, 1, i0:i0 + SW],
                    )
                    nc.gpsimd.partition_all_reduce(
                        m0bc[:, i0:i0 + SW], Amax[:, i0:i0 + SW],
                        128, bass_isa.ReduceOp.max,
                    )
                    for k in range(OG):
                        t = s * OG + k
                        nc.tensor.matmul(
                            ps_m0[:, t:t + 1],
                            lhsT=m0bc[0:1, t * P:(t + 1) * P],
                            rhs=ones_col[0:1, :],
                            start=True, stop=True,
                        )

                # D(s): c2q matmuls (Z column), recip, normalized copy
                # (ACT/DVE split), combined chunk1+2 store
                def phase_D(s):
                    t0 = s * OG
                    for k in range(OG):
                        t = t0 + k
                        ps_cq = ps_cqp.tile([P, LQ + 1], f32, tag="cq")
                        for jc in range(JT):
                            nc.tensor.matmul(
                                ps_cq,
                                lhsT=AT[:, jc, t * P:(t + 1) * P],
                                rhs=qin[:, jc, :],
                                start=(jc == 0), stop=(jc == JT - 1),
                            )
                        nc.vector.reciprocal(invZ[:, t:t + 1], ps_cq[:, D:D + 1])
                        if k % 2 == 0:
                            nc.scalar.activation(
                                g12[:, t, 0:D], ps_cq[:, 0:D], COPY,
                                scale=invZ[:, t:t + 1],
                            )
                        else:
                            nc.vector.tensor_scalar_mul(
                                g12[:, t, 0:D], ps_cq[:, 0:D], invZ[:, t:t + 1]
                            )
                    nc.vector.tensor_mul(
                        g12[:, t0:t0 + OG, D:2 * D], cin[:, t0:t0 + OG, :],
                        g12[:, t0:t0 + OG, 0:D],
                    )
                    nc.sync.dma_start(
                        out12[b, :, t0:t0 + OG, :], g12[:, t0:t0 + OG, :]
                    )

                def tail_q2c():
                    # q2c chain (needs m0 of all 16 tiles)
                    nc.vector.tensor_copy(m016, ps_m0)
                    ebrow = smallp.tile([P, 1], f32, tag="ebrow")
                    nc.vector.reduce_sum(ebrow, m016, axis=AX)
                    ebrow16 = smallp.tile([P, 1], bf16, tag="ebrow16")
                    nc.vector.tensor_copy(ebrow16, ebrow)
                    ps_zb = ps_smp.tile([1, 1], f32, tag="sm")
                    nc.tensor.matmul(
                        ps_zb, lhsT=ebrow16, rhs=ones_col, start=True, stop=True
                    )
                    zb = smallp.tile([1, 1], f32, tag="zbs")
                    nc.vector.tensor_copy(zb, ps_zb)
                    inv_zb = smallp.tile([1, 1], f32, tag="invzb")
                    nc.vector.reciprocal(inv_zb, zb)
                    ps_q2c = ps_smp.tile([1, D], f32, tag="sm")
                    for t in range(T):
                        nc.tensor.matmul(
                            ps_q2c, lhsT=m016[:, t:t + 1], rhs=cin[:, t, :],
                            start=(t == 0), stop=(t == T - 1),
                        )
                    q2cn16 = smallp.tile([1, D], bf16, tag="q2cn")
                    nc.scalar.activation(q2cn16, ps_q2c, COPY, scale=inv_zb)
                    ps_bc = ps_smp.tile([P, D], f32, tag="sm")
                    nc.tensor.matmul(
                        ps_bc, lhsT=ones_row, rhs=q2cn16, start=True, stop=True
                    )
                    q2cb16 = perb.tile([P, D], bf16, tag="q2cb")
                    nc.vector.tensor_copy(q2cb16, ps_bc)
                    return q2cb16

                def tail_g3(q2cb16, s):
                    t0 = s * OG
                    nc.vector.tensor_mul(
                        g3[:, t0:t0 + OG, :], cin[:, t0:t0 + OG, :],
                        q2cb16[:, None, :].to_broadcast((P, OG, D)),
                    )
                    nc.sync.dma_start(
                        out3[b, :, t0:t0 + OG, :], g3[:, t0:t0 + OG, :]
                    )

                return phase_B, phase_C, phase_D, tail_q2c, tail_g3

            # ---- slab-staggered emission; each batch's q2c chain is pulled
            # ahead of its last two c2q slabs; batch 1's S phases interleave
            # with batch 0's tail.
            B0, C0, D0, Q0, G0 = batch_ctx(0)
            B1, C1, D1, Q1, G1 = batch_ctx(1)
            emit_qw(0); B0(0); B0(1); emit_qw(1); C0(0); B0(2); C0(1); D0(0)
            B0(3); C0(2); D0(1); C0(3)
            q2cb0 = Q0()
            D0(2); G0(q2cb0, 0); D0(3); G0(q2cb0, 1)
            B1(0); G0(q2cb0, 2); B1(1); C1(0); G0(q2cb0, 3)
            B1(2); C1(1); D1(0); B1(3); C1(2); D1(1); C1(3)
            q2cb1 = Q1()
            D1(2); G1(q2cb1, 0); G1(q2cb1, 1); D1(3); G1(q2cb1, 2); G1(q2cb1, 3)

    nc.compile()
    return nc


def _get_nc():
    if "nc" not in _cache:
        _cache["nc"] = _build()
    return _cache["nc"]


def run(emb_context, emb_query, W, trace=False, **kwargs):
    import ml_dtypes
    from concourse.bass_utils import run_bass_kernel_spmd

    nc = _get_nc()
    bf = ml_dtypes.bfloat16
    c16 = np.asarray(emb_context, dtype=np.float32).astype(bf)
    # partition-major layouts (pure layout transforms)
    c_pm = np.ascontiguousarray(
        c16.reshape(N, T, P, D).transpose(0, 2, 1, 3)
    )  # (N, P, T, D)
    ct = c16.transpose(0, 2, 1)  # (N, D, LC)
    ct_pm = np.ascontiguousarray(
        ct.reshape(N, DC, P, LC).transpose(0, 2, 1, 3)
    )  # (N, P, DC, LC)
    eq = np.asarray(emb_query, dtype=np.float32).astype(bf)
    q_p = np.concatenate([eq, np.ones((N, LQ, 1), dtype=bf)], axis=2)
    q_pm = np.ascontiguousarray(
        q_p.reshape(N, JT, P, D + 1).transpose(0, 2, 1, 3)
    )  # (N, P, JT, D+1)
    W = np.asarray(W, dtype=np.float32)
    in_maps = [
        {
            "c_pm": np.ascontiguousarray(c_pm[c * NB:(c + 1) * NB]),
            "ct_pm": np.ascontiguousarray(ct_pm[c * NB:(c + 1) * NB]),
            "q_pm": np.ascontiguousarray(q_pm[c * NB:(c + 1) * NB]),
            "W": W,
        }
        for c in range(NCORES)
    ]
    res = run_bass_kernel_spmd(
        nc, in_maps, core_ids=list(range(NCORES)), trace=trace, **kwargs
    )
    # assemble: chunk0 = bf16 context verbatim; device chunks back to
    # row-major (pure layout)
    full = np.empty((N, LC, 4 * D), dtype=np.float32)
    full[:, :, 0:D] = c16.astype(np.float32)
    o12 = np.stack([np.asarray(r["out12"]) for r in res.results])  # (8,NB,P,T,2D)
    o3 = np.stack([np.asarray(r["out3"]) for r in res.results])
    o12 = o12.reshape(N, P, T, 2 * D).transpose(0, 2, 1, 3).reshape(N, LC, 2 * D)
    o3 = o3.reshape(N, P, T, D).transpose(0, 2, 1, 3).reshape(N, LC, D)
    full[:, :, D:3 * D] = o12.astype(np.float32)
    full[:, :, 3 * D:] = o3.astype(np.float32)
    return full, res


def kernel(emb_context, emb_query, W):
    out, _ = run(emb_context, emb_query, W, trace=False)
    return out
